# revision 31
# baseline (speedup 1.0000x reference)
"""MixedFFN Trainium2 kernel (8 NeuronCores, SPMD).

Problem: x [8, 2048, 1024]; shared FFN (W1S [2048,1024], W2S [1024,2048])
applied to positions 0..1984 of every batch; per-position FFN
(W1NS [64,1024,2048], W2NS [64,2048,1024]) applied to positions 1984..2048.
gelu is exact (erf). Output [8, 2048, 1024] fp32.

Sharding:
  - Shared part: data-parallel over batch. Core i computes the shared FFN
    for batch i over the 1984 shared positions using replicated W1S/W2S.
  - Per-position part: sharded over positions. Core i handles positions
    1984+8i .. 1984+8(i+1) for ALL batches, so each NS weight byte is read
    from HBM exactly once across the chip.

Device kernel (per core, identical program, different data):
  - All matmul operands are fp16 (full-rate PE, half the HBM bytes of
    fp32; K<=2048 contractions accumulate in fp32 PSUM so rel err ~1e-3).
  - Shared block rb (512/512/512/448 rows): MM1 phase (per fc: 8 dc-chunk
    matmuls -> PSUM, gelu -> hth[:, fc] fp16), then MM2 phase (per rc,dh:
    16 fc matmuls from hth -> PSUM, copy, DMA out). Phases are dense
    back-to-back PE work so the tensor engine stays at its top DVFS state.
  - NS position p: 8 resident W1N dc-tiles [128, 2048] (one DMA each,
    4KB lines), 4 quarter accumulations [8,512] over dc, gelu -> hsb
    fp16, ONE XBAR DMA-transpose hsb[16,2048] -> hT [128,FC,16] (14ns
    per 16x128 tile, off the PE), then 8 fc-pair W2N tiles [128, 2048]
    with 4 matmuls each accumulating y [8,1024].
  - NS steps are Bresenham-interleaved with shared steps so the NS weight
    stream (64MB/core, the DMA bulk) overlaps shared compute, while PSUM
    fits: ph 2 + py 2 + nsq 2 + pyn 2 = 8 banks.

Host side: shards/casts/packs inputs (numpy), feeds the SPMD run,
reassembles.
"""

import os
import sys

import numpy as np

for _p in ("/opt/trn_rl_repo",):
    if os.path.isdir(_p) and _p not in sys.path:
        sys.path.insert(0, _p)

B, T, D, F, LNS = 8, 2048, 1024, 2048, 64
S = T - LNS  # 1984
NCORES = 8
PPC = LNS // NCORES  # 8 positions per core
DC, FC = D // 128, F // 128  # 8, 16 k-chunks
NRB = 5
RBS = [256, 512, 512, 512, 192]  # row-block sizes covering the S=1984 rows
RBOFF = [0, 256, 768, 1280, 1792]  # block 0 small: the first matmul only
# needs 1.25MB of DMA (xt0 + first W1TP pair) instead of 2.25MB
XTLEN = DC * S  # free length of the packed x tensor
PB = 16  # padded batch stride for 32B-aligned fp16 stationary slices

LAST_RESULTS = None  # BassKernelResults of the most recent run (for test.py)

_cached = None


def _interleave(a, b):
    """Merge two step lists proportionally (Bresenham); each step is a
    zero-arg callable that emits instructions."""
    if not b:
        return list(a)
    if not a:
        return list(b)
    out = []
    ia = ib = 0
    na, nb = len(a), len(b)
    while ia < na or ib < nb:
        if ib * na <= ia * nb:
            if ib < nb:
                out.append(b[ib])
                ib += 1
            else:
                out.append(a[ia])
                ia += 1
        else:
            if ia < na:
                out.append(a[ia])
                ia += 1
            else:
                out.append(b[ib])
                ib += 1
    return out


def _build():
    import concourse.tile as tile
    from concourse import bacc
    from concourse import mybir

    f32 = mybir.dt.float32
    f32r = mybir.dt.float32r
    f16 = mybir.dt.float16
    GELU = mybir.ActivationFunctionType.Gelu
    W1N_BUFS = int(os.environ.get("MIXEDFFN_W1N_BUFS", "10"))
    W2N_BUFS = int(os.environ.get("MIXEDFFN_W2N_BUFS", "6"))

    nc = bacc.Bacc("TRN2", target_bir_lowering=False, debug=False, num_devices=NCORES)

    XT = nc.dram_tensor("XT", [128, XTLEN], f16, kind="ExternalInput").ap()
    XNS = nc.dram_tensor("XNS", [128, DC * PPC * PB], f16, kind="ExternalInput").ap()
    W1TP = nc.dram_tensor("W1TP", [FC // 2, 128, 2 * DC * 128], f16, kind="ExternalInput").ap()
    W2TP = nc.dram_tensor("W2TP", [FC // 2, 128, 2 * D], f16, kind="ExternalInput").ap()
    W1N = nc.dram_tensor("W1N", [PPC, D, F], f16, kind="ExternalInput").ap()
    W2NP = nc.dram_tensor("W2NP", [PPC, FC // 2, 128, 2 * D], f16, kind="ExternalInput").ap()
    YS = nc.dram_tensor("YS", [S, D], f32, kind="ExternalOutput").ap()
    YN = nc.dram_tensor("YN", [PPC, B, D], f32, kind="ExternalOutput").ap()

    with tile.TileContext(nc) as tc:
        with (
            tc.tile_pool(name="wres", bufs=1) as wres,
            tc.tile_pool(name="xt", bufs=2) as xtp,
            tc.tile_pool(name="hth", bufs=2) as hthp,
            tc.tile_pool(name="ysb", bufs=2) as ysbp,
            tc.tile_pool(name="w1n", bufs=W1N_BUFS) as w1np,
            tc.tile_pool(name="w2n", bufs=W2N_BUFS) as w2np,
            tc.tile_pool(name="hns", bufs=2) as hnsp,
            tc.tile_pool(name="htns", bufs=2) as htnsp,
            tc.tile_pool(name="ph", bufs=2, space="PSUM") as php,
            tc.tile_pool(name="py", bufs=2, space="PSUM") as pyp,
            tc.tile_pool(name="nsq", bufs=2, space="PSUM") as nsqp,
            tc.tile_pool(name="pyn", bufs=1, space="PSUM") as pynp,
        ):
            # ---- resident tiles; weight loads are emitted as interleaved
            # steps so the first matmuls start after ~1MB, not 8MB ----
            w1t_sb = wres.tile([128, FC, DC, 128], f16)
            w2t_sb = wres.tile([128, FC, D], f16)
            xns_sb = wres.tile([128, DC * PPC * PB], f16)

            def xns_load():
                nc.sync.dma_start(out=xns_sb[:], in_=XNS[:])

            def w_load_steps():
                def w1_step(j):
                    def step():
                        nc.sync.dma_start(
                            out=w1t_sb[:, 2 * j : 2 * j + 2], in_=W1TP[j]
                        )

                    return step

                def w2_step(j):
                    def step():
                        nc.sync.dma_start(
                            out=w2t_sb[:, 2 * j : 2 * j + 2, :], in_=W2TP[j]
                        )

                    return step

                # all W1 pairs first (MM1 phase), then W2 pairs (MM2 phase)
                return [w1_step(j) for j in range(FC // 2)] + [
                    w2_step(j) for j in range(FC // 2)
                ]

            # ---- step generators ----
            state = {}

            def xt_load(rb):
                def step():
                    rbsz = RBS[rb]
                    xt = xtp.tile([128, DC, rbsz], f16, name=f"xtt{rb}", tag="xtt")
                    off = DC * RBOFF[rb]
                    nc.sync.dma_start(out=xt[:], in_=XT[:, off : off + DC * rbsz])
                    state[("xt", rb)] = xt

                return step

            def shared_steps(rb):
                rbsz = RBS[rb]
                steps = [xt_load(rb)] if rb == 0 else []

                def mm1_step(fc):
                    def step():
                        xt = state[("xt", rb)]
                        if fc == 0:
                            state[("hth", rb)] = hthp.tile(
                                [128, FC, rbsz], f16, name=f"hth{rb}", tag="hth"
                            )
                        hth = state[("hth", rb)]
                        ph = php.tile([128, rbsz], f32, name=f"ph{rb}_{fc}", tag="ph")
                        for dc in range(DC):
                            nc.tensor.matmul(
                                ph[:],
                                w1t_sb[:, fc, dc, :],
                                xt[:, dc, :],
                                start=(dc == 0),
                                stop=(dc == DC - 1),
                                skip_group_check=True,
                            )
                        nc.scalar.activation(hth[:, fc, :], ph[:], GELU)

                    return step

                def mm2_step(rc, dh):
                    def step():
                        hth = state[("hth", rb)]
                        nrows = min(128, rbsz - rc * 128)
                        py = pyp.tile(
                            [nrows, 512], f32, name=f"py{rb}_{rc}_{dh}", tag="py"
                        )
                        for fc in range(FC):
                            nc.tensor.matmul(
                                py[:],
                                hth[:, fc, rc * 128 : rc * 128 + nrows],
                                w2t_sb[:, fc, dh * 512 : (dh + 1) * 512],
                                start=(fc == 0),
                                stop=(fc == FC - 1),
                                skip_group_check=True,
                            )
                        if dh == 0:
                            state[("ysb", rb, rc)] = ysbp.tile(
                                [nrows, D], f32, name=f"ysb{rb}_{rc}", tag="ysb"
                            )
                        ysb = state[("ysb", rb, rc)]
                        nc.vector.tensor_copy(
                            ysb[:, dh * 512 : (dh + 1) * 512], py[:]
                        )
                        if dh == 1:
                            row0 = RBOFF[rb] + rc * 128
                            nc.sync.dma_start(
                                out=YS[row0 : row0 + nrows, :], in_=ysb[:]
                            )

                    return step

                m1 = steps + [mm1_step(fc) for fc in range(FC)]
                nrc = (rbsz + 127) // 128
                m2 = [mm2_step(rc, dh) for rc in range(nrc) for dh in range(2)]
                return m1, m2

            def ns_steps(p):
                """Steps for one NS position: 8 W1 loads + 4 quarter-MM1 +
                8 transpose pairs + 8 fc-pair MM2."""
                steps = []

                def w1_load(dc):
                    def step():
                        w1 = w1np.tile(
                            [128, F], f16, name=f"w1_{p}_{dc}", tag="w1"
                        )
                        nc.sync.dma_start(
                            out=w1[:], in_=W1N[p, dc * 128 : (dc + 1) * 128, :]
                        )
                        state[("w1n", p, dc)] = w1

                    return step

                def mm1_step(q):
                    def step():
                        if q == 0:
                            # 16 partitions for the XBAR transpose tile; rows
                            # 8:16 are never written and never consumed.
                            state[("hsb", p)] = hnsp.tile(
                                [16, F], f16, name=f"hsb{p}", tag="hsb"
                            )
                            state[("hT", p)] = htnsp.tile(
                                [128, FC, 16], f16, name=f"hT{p}", tag="hT"
                            )
                        phq = nsqp.tile(
                            [B, 512], f32, name=f"phq{p}_{q}", tag="nsq"
                        )
                        for dc in range(DC):
                            nc.tensor.matmul(
                                phq[:],
                                xns_sb[
                                    :,
                                    dc * PPC * PB
                                    + p * PB : dc * PPC * PB
                                    + p * PB
                                    + B,
                                ],
                                state[("w1n", p, dc)][:, q * 512 : (q + 1) * 512],
                                start=(dc == 0),
                                stop=(dc == DC - 1),
                                skip_group_check=True,
                            )
                        nc.scalar.activation(
                            state[("hsb", p)][0:B, q * 512 : (q + 1) * 512], phq[:], GELU
                        )

                    return step

                def tr_step():
                    def step():
                        # XBAR: tile t of hsb[16, t*128:(t+1)*128] lands in
                        # hT[:, t, :]; issued on the ACT ring after the gelus.
                        nc.scalar.dma_start_transpose(
                            state[("hT", p)][:, :, :], state[("hsb", p)][0:16, :]
                        )

                    return step

                def mm2_step(j):
                    def step():
                        if j == 0:
                            state[("pyn", p)] = pynp.tile(
                                [B, D], f32, name=f"pyn{p}", tag="pyn"
                            )
                        pyn = state[("pyn", p)]
                        hT = state[("hT", p)]
                        w2 = w2np.tile([128, 2 * D], f16, name=f"w2_{p}_{j}", tag="w2")
                        nc.sync.dma_start(out=w2[:], in_=W2NP[p, j])
                        for jj in range(2):
                            fc = 2 * j + jj
                            for dh in range(2):
                                nc.tensor.matmul(
                                    pyn[:, dh * 512 : (dh + 1) * 512],
                                    hT[:, fc, 0:B],
                                    w2[:, jj * D + dh * 512 : jj * D + (dh + 1) * 512],
                                    start=(fc == 0),
                                    stop=(fc == FC - 1),
                                    skip_group_check=True,
                                )
                        if j == FC // 2 - 1:
                            ysb = ysbp.tile([B, D], f32, name=f"ysbn{p}", tag="ysb")
                            nc.vector.tensor_copy(ysb[:], pyn[:])
                            nc.sync.dma_start(out=YN[p], in_=ysb[:])

                    return step

                steps += [w1_load(dc) for dc in range(DC)]
                steps += [mm1_step(q) for q in range(4)]
                steps += [tr_step()]
                steps += [mm2_step(j) for j in range(FC // 2)]
                return steps

            # ---- emission ----
            # Shared blocks are phase-pipelined: M1(0) M1(1) M2(0) M1(2)
            # M2(1) M1(3) M2(2) M2(3), so M2(0) (which needs the full W2T
            # resident) has two MM1 phases of PE work ahead of it while
            # W2TP streams in. The NS stream is software-pipelined the same
            # way -- position p's MM2 segment is woven into position p+1's
            # load/MM1 segment so the act->XBAR-transpose->hT chain latency
            # is hidden -- and merged ~2:1 into the shared step list. The
            # warmup segment (M1(0) + resident weight loads) carries no NS
            # steps: early DMA bandwidth all goes to xt0/W1TP/W2TP/xt1.
            m1s, m2s = {}, {}
            for rb in range(NRB):
                m1s[rb], m2s[rb] = shared_steps(rb)

            wl = w_load_steps()  # [w1p 0..7, w2p 0..7]
            warm = [m1s[0][0], wl[0], xns_load]  # xt0, w1 pair 0, xns
            rest_wl = wl[1:]
            for k, st in enumerate(m1s[0][1:]):
                warm.append(st)
                warm += rest_wl[2 * k : 2 * k + 2]
                if k == 2:
                    warm.append(xt_load(1))

            sh_rest = (
                m1s[1]
                + [xt_load(2)]
                + m2s[0]
                + m1s[2]
                + [xt_load(3)]
                + m2s[1]
                + m1s[3]
                + [xt_load(4)]
                + m2s[2]
                + m1s[4]
                + m2s[3]
                + m2s[4]
            )
            # NS: A(p) = loads (8) + mm1+tr (5), B(p) = mm2 (8 steps); B(p)
            # rides in the SECOND part of A(p+1) (after the loads), so the
            # gelu -> XBAR-transpose -> hT chain of p has ~the whole load
            # phase of p+1 (plus woven shared steps) to complete before
            # B(p)'s stationary loads consume hT.
            segs = []
            for p in range(PPC):
                st = ns_steps(p)
                segs.append((st[:8], st[8:13], st[13:]))
            ns_flat = list(segs[0][0]) + list(segs[0][1])
            for p in range(1, PPC):
                ns_flat += segs[p][0]
                ns_flat += _interleave(segs[p][1], segs[p - 1][2])
            ns_flat += segs[PPC - 1][2]

            for st in warm:
                st()
            # Finish the NS stream before the shared tail: the kernel then
            # ends with dense, dependency-free shared MM2 work.
            for st in _interleave(sh_rest[:-10], ns_flat) + sh_rest[-10:]:
                st()

    nc.compile()
    return nc


def _prepare_inputs(x, W1S, W2S, W1NS, W2NS):
    x = np.asarray(x, dtype=np.float32)
    # W1TP [FC//2, 128, 2, DC, 128]: fc-pair-packed blocks of W1S.T
    w1t = (
        np.asarray(W1S.T, dtype=np.float16)
        .reshape(DC, 128, FC, 128)
        .transpose(2, 1, 0, 3)
    )  # [FC, 128, DC, 128]
    w1tp = np.ascontiguousarray(
        w1t.reshape(FC // 2, 2, 128, DC, 128)
        .transpose(0, 2, 1, 3, 4)
        .reshape(FC // 2, 128, 2 * DC * 128)
    )
    # W2TP [FC//2, 128, 2*D]: fc-pair-packed chunks of W2S.T
    w2t = np.asarray(W2S.T, dtype=np.float16).reshape(FC // 2, 2, 128, D)
    w2tp = np.ascontiguousarray(
        w2t.transpose(0, 2, 1, 3).reshape(FC // 2, 128, 2 * D)
    )
    in_maps = []
    for i in range(NCORES):
        # XT [128, DC*S]: concatenated row blocks, partition-major
        xt = np.empty((128, XTLEN), dtype=np.float16)
        for rb in range(NRB):
            rbsz = RBS[rb]
            blk = (
                x[i][RBOFF[rb] : RBOFF[rb] + rbsz, :]
                .T.reshape(DC, 128, rbsz)
                .transpose(1, 0, 2)
                .reshape(128, DC * rbsz)
            )
            off = DC * RBOFF[rb]
            xt[:, off : off + DC * rbsz] = blk
        xi = x[:, S + PPC * i : S + PPC * (i + 1), :]  # [B, PPC, D]
        # [128, dc, p, b] flattened to [128, dc*p*PB]
        xns4 = (
            xi.transpose(2, 1, 0)  # [D, PPC, B]
            .reshape(DC, 128, PPC, B)
            .transpose(1, 0, 2, 3)
            .astype(np.float16)
        )  # [128, DC, PPC, B]
        xns = np.zeros((128, DC, PPC, PB), dtype=np.float16)
        xns[:, :, :, :B] = xns4
        xns = np.ascontiguousarray(xns.reshape(128, DC * PPC * PB))
        w2n = (
            np.asarray(W2NS[PPC * i : PPC * (i + 1)], dtype=np.float16)
            .reshape(PPC, FC // 2, 2, 128, D)
            .transpose(0, 1, 3, 2, 4)
            .reshape(PPC, FC // 2, 128, 2 * D)
        )
        in_maps.append(
            {
                "XT": np.ascontiguousarray(xt),
                "XNS": xns,
                "W1TP": w1tp,
                "W2TP": w2tp,
                "W1N": np.ascontiguousarray(
                    W1NS[PPC * i : PPC * (i + 1)].astype(np.float16)
                ),
                "W2NP": np.ascontiguousarray(w2n),
            }
        )
    return in_maps


def kernel(x, W1S, W2S, W1NS, W2NS):
    global _cached, LAST_RESULTS
    from concourse.bass_utils import run_bass_kernel_spmd

    if _cached is None:
        _cached = _build()
    nc = _cached
    in_maps = _prepare_inputs(x, W1S, W2S, W1NS, W2NS)
    trace = bool(os.environ.get("MIXEDFFN_TRACE"))
    res = run_bass_kernel_spmd(
        nc, in_maps, core_ids=list(range(NCORES)), trace=trace
    )
    LAST_RESULTS = res
    out = np.empty((B, T, D), dtype=np.float32)
    for i in range(NCORES):
        out[i, :S, :] = res.results[i]["YS"]
        yn = res.results[i]["YN"]  # [PPC, B, D]
        for p in range(PPC):
            out[:, S + PPC * i + p, :] = yn[p]
    return out


# revision 34
# speedup vs baseline: 1.0738x; 1.0738x over previous
"""MixedFFN Trainium2 kernel (8 NeuronCores, SPMD).

Problem: x [8, 2048, 1024]; shared FFN (W1S [2048,1024], W2S [1024,2048])
applied to positions 0..1984 of every batch; per-position FFN
(W1NS [64,1024,2048], W2NS [64,2048,1024]) applied to positions 1984..2048.
gelu is exact (erf). Output [8, 2048, 1024] fp32.

Sharding:
  - Shared part: data-parallel over batch. Core i computes the shared FFN
    for batch i over the 1984 shared positions using replicated W1S/W2S.
  - Per-position part: sharded over positions. Core i handles positions
    1984+8i .. 1984+8(i+1) for ALL batches, so each NS weight byte is read
    from HBM exactly once across the chip.

Device kernel (per core, identical program, different data):
  - All matmul operands are fp16 (full-rate PE, half the HBM bytes of
    fp32; K<=2048 contractions accumulate in fp32 PSUM so rel err ~1e-3).
  - Shared block rb (512/512/512/448 rows): MM1 phase (per fc: 8 dc-chunk
    matmuls -> PSUM, gelu -> hth[:, fc] fp16), then MM2 phase (per rc,dh:
    16 fc matmuls from hth -> PSUM, copy, DMA out). Phases are dense
    back-to-back PE work so the tensor engine stays at its top DVFS state.
  - NS position p: 8 resident W1N dc-tiles [128, 2048] (one DMA each,
    4KB lines), 4 quarter accumulations [8,512] over dc, gelu -> hsb
    fp16, ONE XBAR DMA-transpose hsb[16,2048] -> hT [128,FC,16] (14ns
    per 16x128 tile, off the PE), then 8 fc-pair W2N tiles [128, 2048]
    with 4 matmuls each accumulating y [8,1024].
  - Scheduling: shared blocks are phase-pipelined (M1(0) M1(1) M2(0)
    M1(2) ...) so MM2's W2T resident dependency has two MM1 phases of PE
    work in front of it; the NS stream is software-pipelined (position
    p's MM2 rides inside position p+1's load/MM1 segment, hiding the
    gelu->XBAR->hT latency) and Bresenham-woven ~2:1 into the shared
    steps so the 64MB/core NS weight stream overlaps shared compute.
    PSUM: ph 2 + py 2 + nsq 2 + pyn 2 = 8 banks. Measured ~370us
    (tensor-bound: ~778k matmul rows at 2.4GHz x 0.925 util throttle,
    92%+ tensor occupancy).

Host side: shards/casts/packs inputs (numpy), feeds the SPMD run,
reassembles.
"""

import os
import sys

import numpy as np

for _p in ("/opt/trn_rl_repo",):
    if os.path.isdir(_p) and _p not in sys.path:
        sys.path.insert(0, _p)

B, T, D, F, LNS = 8, 2048, 1024, 2048, 64
S = T - LNS  # 1984
NCORES = 8
PPC = LNS // NCORES  # 8 positions per core
DC, FC = D // 128, F // 128  # 8, 16 k-chunks
NRB = 4
RBS = [512, 512, 512, 448]  # row-block sizes covering the S=1984 rows
RBOFF = [0, 512, 1024, 1536]
XTLEN = DC * S  # free length of the packed x tensor
PB = 16  # padded batch stride for 32B-aligned fp16 stationary slices

LAST_RESULTS = None  # BassKernelResults of the most recent run (for test.py)

_cached = None


def _interleave(a, b):
    """Merge two step lists proportionally (Bresenham); each step is a
    zero-arg callable that emits instructions."""
    if not b:
        return list(a)
    if not a:
        return list(b)
    out = []
    ia = ib = 0
    na, nb = len(a), len(b)
    while ia < na or ib < nb:
        if ib * na <= ia * nb:
            if ib < nb:
                out.append(b[ib])
                ib += 1
            else:
                out.append(a[ia])
                ia += 1
        else:
            if ia < na:
                out.append(a[ia])
                ia += 1
            else:
                out.append(b[ib])
                ib += 1
    return out


def _build():
    import concourse.tile as tile
    from concourse import bacc
    from concourse import mybir

    f32 = mybir.dt.float32
    f32r = mybir.dt.float32r
    f16 = mybir.dt.float16
    GELU = mybir.ActivationFunctionType.Gelu
    W1N_BUFS = int(os.environ.get("MIXEDFFN_W1N_BUFS", "10"))
    W2N_BUFS = int(os.environ.get("MIXEDFFN_W2N_BUFS", "6"))

    nc = bacc.Bacc("TRN2", target_bir_lowering=False, debug=False, num_devices=NCORES)

    XT = nc.dram_tensor("XT", [128, XTLEN], f16, kind="ExternalInput").ap()
    XNS = nc.dram_tensor("XNS", [128, DC * PPC * PB], f16, kind="ExternalInput").ap()
    W1TP = nc.dram_tensor("W1TP", [FC // 2, 128, 2 * DC * 128], f16, kind="ExternalInput").ap()
    W2TP = nc.dram_tensor("W2TP", [FC // 2, 128, 2 * D], f16, kind="ExternalInput").ap()
    W1N = nc.dram_tensor("W1N", [PPC, D, F], f16, kind="ExternalInput").ap()
    W2NP = nc.dram_tensor("W2NP", [PPC, FC // 2, 128, 2 * D], f16, kind="ExternalInput").ap()
    YS = nc.dram_tensor("YS", [S, D], f32, kind="ExternalOutput").ap()
    YN = nc.dram_tensor("YN", [PPC, B, D], f32, kind="ExternalOutput").ap()

    with tile.TileContext(nc) as tc:
        with (
            tc.tile_pool(name="wres", bufs=1) as wres,
            tc.tile_pool(name="xt", bufs=2) as xtp,
            tc.tile_pool(name="hth", bufs=2) as hthp,
            tc.tile_pool(name="ysb", bufs=2) as ysbp,
            tc.tile_pool(name="w1n", bufs=W1N_BUFS) as w1np,
            tc.tile_pool(name="w2n", bufs=W2N_BUFS) as w2np,
            tc.tile_pool(name="hns", bufs=2) as hnsp,
            tc.tile_pool(name="htns", bufs=2) as htnsp,
            tc.tile_pool(name="ph", bufs=2, space="PSUM") as php,
            tc.tile_pool(name="py", bufs=2, space="PSUM") as pyp,
            tc.tile_pool(name="nsq", bufs=2, space="PSUM") as nsqp,
            tc.tile_pool(name="pyn", bufs=1, space="PSUM") as pynp,
        ):
            # ---- resident tiles; weight loads are emitted as interleaved
            # steps so the first matmuls start after ~1MB, not 8MB ----
            w1t_sb = wres.tile([128, FC, DC, 128], f16)
            w2t_sb = wres.tile([128, FC, D], f16)
            xns_sb = wres.tile([128, DC * PPC * PB], f16)

            def xns_load():
                nc.sync.dma_start(out=xns_sb[:], in_=XNS[:])

            def w_load_steps():
                def w1_step(j):
                    def step():
                        nc.sync.dma_start(
                            out=w1t_sb[:, 2 * j : 2 * j + 2], in_=W1TP[j]
                        )

                    return step

                def w2_step(j):
                    def step():
                        nc.sync.dma_start(
                            out=w2t_sb[:, 2 * j : 2 * j + 2, :], in_=W2TP[j]
                        )

                    return step

                # all W1 pairs first (MM1 phase), then W2 pairs (MM2 phase)
                return [w1_step(j) for j in range(FC // 2)] + [
                    w2_step(j) for j in range(FC // 2)
                ]

            # ---- step generators ----
            state = {}

            def xt_load(rb):
                def step():
                    rbsz = RBS[rb]
                    xt = xtp.tile([128, DC, rbsz], f16, name=f"xtt{rb}", tag="xtt")
                    off = DC * RBOFF[rb]
                    nc.sync.dma_start(out=xt[:], in_=XT[:, off : off + DC * rbsz])
                    state[("xt", rb)] = xt

                return step

            def shared_steps(rb):
                rbsz = RBS[rb]
                steps = [xt_load(rb)] if rb == 0 else []

                def mm1_step(fc):
                    def step():
                        xt = state[("xt", rb)]
                        if fc == 0:
                            state[("hth", rb)] = hthp.tile(
                                [128, FC, rbsz], f16, name=f"hth{rb}", tag="hth"
                            )
                        hth = state[("hth", rb)]
                        ph = php.tile([128, rbsz], f32, name=f"ph{rb}_{fc}", tag="ph")
                        for dc in range(DC):
                            nc.tensor.matmul(
                                ph[:],
                                w1t_sb[:, fc, dc, :],
                                xt[:, dc, :],
                                start=(dc == 0),
                                stop=(dc == DC - 1),
                                skip_group_check=True,
                            )
                        nc.scalar.activation(hth[:, fc, :], ph[:], GELU)

                    return step

                def mm2_step(rc, dh):
                    def step():
                        hth = state[("hth", rb)]
                        nrows = min(128, rbsz - rc * 128)
                        py = pyp.tile(
                            [nrows, 512], f32, name=f"py{rb}_{rc}_{dh}", tag="py"
                        )
                        for fc in range(FC):
                            nc.tensor.matmul(
                                py[:],
                                hth[:, fc, rc * 128 : rc * 128 + nrows],
                                w2t_sb[:, fc, dh * 512 : (dh + 1) * 512],
                                start=(fc == 0),
                                stop=(fc == FC - 1),
                                skip_group_check=True,
                            )
                        if dh == 0:
                            state[("ysb", rb, rc)] = ysbp.tile(
                                [nrows, D], f32, name=f"ysb{rb}_{rc}", tag="ysb"
                            )
                        ysb = state[("ysb", rb, rc)]
                        nc.vector.tensor_copy(
                            ysb[:, dh * 512 : (dh + 1) * 512], py[:]
                        )
                        if dh == 1:
                            row0 = RBOFF[rb] + rc * 128
                            nc.sync.dma_start(
                                out=YS[row0 : row0 + nrows, :], in_=ysb[:]
                            )

                    return step

                m1 = steps + [mm1_step(fc) for fc in range(FC)]
                nrc = (rbsz + 127) // 128
                m2 = [mm2_step(rc, dh) for rc in range(nrc) for dh in range(2)]
                return m1, m2

            def ns_steps(p):
                """Steps for one NS position: 8 W1 loads + 4 quarter-MM1 +
                8 transpose pairs + 8 fc-pair MM2."""
                steps = []

                def w1_load(dc):
                    def step():
                        w1 = w1np.tile(
                            [128, F], f16, name=f"w1_{p}_{dc}", tag="w1"
                        )
                        nc.sync.dma_start(
                            out=w1[:], in_=W1N[p, dc * 128 : (dc + 1) * 128, :]
                        )
                        state[("w1n", p, dc)] = w1

                    return step

                def mm1_step(q):
                    def step():
                        if q == 0:
                            # 16 partitions for the XBAR transpose tile; rows
                            # 8:16 are never written and never consumed.
                            state[("hsb", p)] = hnsp.tile(
                                [16, F], f16, name=f"hsb{p}", tag="hsb"
                            )
                            state[("hT", p)] = htnsp.tile(
                                [128, FC, 16], f16, name=f"hT{p}", tag="hT"
                            )
                        phq = nsqp.tile(
                            [B, 512], f32, name=f"phq{p}_{q}", tag="nsq"
                        )
                        for dc in range(DC):
                            nc.tensor.matmul(
                                phq[:],
                                xns_sb[
                                    :,
                                    dc * PPC * PB
                                    + p * PB : dc * PPC * PB
                                    + p * PB
                                    + B,
                                ],
                                state[("w1n", p, dc)][:, q * 512 : (q + 1) * 512],
                                start=(dc == 0),
                                stop=(dc == DC - 1),
                                skip_group_check=True,
                            )
                        nc.scalar.activation(
                            state[("hsb", p)][0:B, q * 512 : (q + 1) * 512], phq[:], GELU
                        )

                    return step

                def tr_step():
                    def step():
                        # XBAR: tile t of hsb[16, t*128:(t+1)*128] lands in
                        # hT[:, t, :]; issued on the ACT ring after the gelus.
                        nc.scalar.dma_start_transpose(
                            state[("hT", p)][:, :, :], state[("hsb", p)][0:16, :]
                        )

                    return step

                def mm2_step(j):
                    def step():
                        if j == 0:
                            state[("pyn", p)] = pynp.tile(
                                [B, D], f32, name=f"pyn{p}", tag="pyn"
                            )
                        pyn = state[("pyn", p)]
                        hT = state[("hT", p)]
                        w2 = w2np.tile([128, 2 * D], f16, name=f"w2_{p}_{j}", tag="w2")
                        nc.sync.dma_start(out=w2[:], in_=W2NP[p, j])
                        for jj in range(2):
                            fc = 2 * j + jj
                            for dh in range(2):
                                nc.tensor.matmul(
                                    pyn[:, dh * 512 : (dh + 1) * 512],
                                    hT[:, fc, 0:B],
                                    w2[:, jj * D + dh * 512 : jj * D + (dh + 1) * 512],
                                    start=(fc == 0),
                                    stop=(fc == FC - 1),
                                    skip_group_check=True,
                                )
                        if j == FC // 2 - 1:
                            ysb = ysbp.tile([B, D], f32, name=f"ysbn{p}", tag="ysb")
                            nc.vector.tensor_copy(ysb[:], pyn[:])
                            nc.sync.dma_start(out=YN[p], in_=ysb[:])

                    return step

                steps += [w1_load(dc) for dc in range(DC)]
                steps += [mm1_step(q) for q in range(4)]
                steps += [tr_step()]
                steps += [mm2_step(j) for j in range(FC // 2)]
                return steps

            # ---- emission ----
            # Shared blocks are phase-pipelined: M1(0) M1(1) M2(0) M1(2)
            # M2(1) M1(3) M2(2) M2(3), so M2(0) (which needs the full W2T
            # resident) has two MM1 phases of PE work ahead of it while
            # W2TP streams in. The NS stream is software-pipelined the same
            # way -- position p's MM2 segment is woven into position p+1's
            # load/MM1 segment so the act->XBAR-transpose->hT chain latency
            # is hidden -- and merged ~2:1 into the shared step list. The
            # warmup segment (M1(0) + resident weight loads) carries no NS
            # steps: early DMA bandwidth all goes to xt0/W1TP/W2TP/xt1.
            m1s, m2s = {}, {}
            for rb in range(NRB):
                m1s[rb], m2s[rb] = shared_steps(rb)

            wl = w_load_steps()  # [w1p 0..7, w2p 0..7]
            warm = [m1s[0][0], wl[0], xns_load]  # xt0, w1 pair 0, xns
            rest_wl = wl[1:]
            for k, st in enumerate(m1s[0][1:]):
                warm.append(st)
                warm += rest_wl[2 * k : 2 * k + 2]
                if k == 2:
                    warm.append(xt_load(1))

            sh_rest = (
                m1s[1]
                + [xt_load(2)]
                + m2s[0]
                + m1s[2]
                + [xt_load(3)]
                + m2s[1]
                + m1s[3]
                + m2s[2]
                + m2s[3]
            )
            # NS: A(p) = loads (8) + mm1+tr (5), B(p) = mm2 (8 steps); B(p)
            # rides in the SECOND part of A(p+1) (after the loads), so the
            # gelu -> XBAR-transpose -> hT chain of p has ~the whole load
            # phase of p+1 (plus woven shared steps) to complete before
            # B(p)'s stationary loads consume hT.
            segs = []
            for p in range(PPC):
                st = ns_steps(p)
                segs.append((st[:8], st[8:13], st[13:]))
            ns_flat = list(segs[0][0]) + list(segs[0][1])
            for p in range(1, PPC):
                ns_flat += segs[p][0]
                ns_flat += _interleave(segs[p][1], segs[p - 1][2])
            ns_flat += segs[PPC - 1][2]

            for st in warm:
                st()
            # Finish the NS stream before the shared tail: the kernel then
            # ends with dense, dependency-free shared MM2 work.
            for st in _interleave(sh_rest[:-10], ns_flat) + sh_rest[-10:]:
                st()

    nc.compile()
    return nc


def _prepare_inputs(x, W1S, W2S, W1NS, W2NS):
    x = np.asarray(x, dtype=np.float32)
    # W1TP [FC//2, 128, 2, DC, 128]: fc-pair-packed blocks of W1S.T
    w1t = (
        np.asarray(W1S.T, dtype=np.float16)
        .reshape(DC, 128, FC, 128)
        .transpose(2, 1, 0, 3)
    )  # [FC, 128, DC, 128]
    w1tp = np.ascontiguousarray(
        w1t.reshape(FC // 2, 2, 128, DC, 128)
        .transpose(0, 2, 1, 3, 4)
        .reshape(FC // 2, 128, 2 * DC * 128)
    )
    # W2TP [FC//2, 128, 2*D]: fc-pair-packed chunks of W2S.T
    w2t = np.asarray(W2S.T, dtype=np.float16).reshape(FC // 2, 2, 128, D)
    w2tp = np.ascontiguousarray(
        w2t.transpose(0, 2, 1, 3).reshape(FC // 2, 128, 2 * D)
    )
    in_maps = []
    for i in range(NCORES):
        # XT [128, DC*S]: concatenated row blocks, partition-major
        xt = np.empty((128, XTLEN), dtype=np.float16)
        for rb in range(NRB):
            rbsz = RBS[rb]
            blk = (
                x[i][RBOFF[rb] : RBOFF[rb] + rbsz, :]
                .T.reshape(DC, 128, rbsz)
                .transpose(1, 0, 2)
                .reshape(128, DC * rbsz)
            )
            off = DC * RBOFF[rb]
            xt[:, off : off + DC * rbsz] = blk
        xi = x[:, S + PPC * i : S + PPC * (i + 1), :]  # [B, PPC, D]
        # [128, dc, p, b] flattened to [128, dc*p*PB]
        xns4 = (
            xi.transpose(2, 1, 0)  # [D, PPC, B]
            .reshape(DC, 128, PPC, B)
            .transpose(1, 0, 2, 3)
            .astype(np.float16)
        )  # [128, DC, PPC, B]
        xns = np.zeros((128, DC, PPC, PB), dtype=np.float16)
        xns[:, :, :, :B] = xns4
        xns = np.ascontiguousarray(xns.reshape(128, DC * PPC * PB))
        w2n = (
            np.asarray(W2NS[PPC * i : PPC * (i + 1)], dtype=np.float16)
            .reshape(PPC, FC // 2, 2, 128, D)
            .transpose(0, 1, 3, 2, 4)
            .reshape(PPC, FC // 2, 128, 2 * D)
        )
        in_maps.append(
            {
                "XT": np.ascontiguousarray(xt),
                "XNS": xns,
                "W1TP": w1tp,
                "W2TP": w2tp,
                "W1N": np.ascontiguousarray(
                    W1NS[PPC * i : PPC * (i + 1)].astype(np.float16)
                ),
                "W2NP": np.ascontiguousarray(w2n),
            }
        )
    return in_maps


def kernel(x, W1S, W2S, W1NS, W2NS):
    global _cached, LAST_RESULTS
    from concourse.bass_utils import run_bass_kernel_spmd

    if _cached is None:
        _cached = _build()
    nc = _cached
    in_maps = _prepare_inputs(x, W1S, W2S, W1NS, W2NS)
    trace = bool(os.environ.get("MIXEDFFN_TRACE"))
    res = run_bass_kernel_spmd(
        nc, in_maps, core_ids=list(range(NCORES)), trace=trace
    )
    LAST_RESULTS = res
    out = np.empty((B, T, D), dtype=np.float32)
    for i in range(NCORES):
        out[i, :S, :] = res.results[i]["YS"]
        yn = res.results[i]["YN"]  # [PPC, B, D]
        for p in range(PPC):
            out[:, S + PPC * i + p, :] = yn[p]
    return out
